# revision 12
# baseline (speedup 1.0000x reference)
"""Trainium2 Bass kernel for nn_CNNndpPolicy (CNN-NDP policy head + DMP rollout).

Strategy
--------
Pure data parallel over batch: each of the 8 NeuronCores processes B/8 = 2048
batch rows (M/8 = 4096 DMP channels).

The expensive part of the reference is a 300-step Euler rollout of the DMP
transformation system. Key algebraic facts exploited here:

1. The phase x_t = (1-DT)^t is input-independent, so the RBF basis
   Phi_t = psi(x_t) * x_t / sum(psi(x_t)) is a [300, 32] constant computed on
   the host. The forcing term for all steps is ONE matmul F = w @ Phi^T.
2. The output `a` is a first difference of Y, and y_{t+1} - y_t = DT*z_t,
   so a[:, t] = zs_t with zs = DT*z. Y itself is never materialized.
3. Eliminating y yields a second-order linear recurrence in zs:
       zs_{t+2} = 2*lam*zs_{t+1} - lam^2*zs_t + (U_{t+1} - U_t)
   with lam = 1 - DT*a_z/2 (critically damped: b_z = a_z/4 makes the
   characteristic discriminant exactly zero). This factors into TWO
   first-order scans (e-scan, zs-scan), each a single native
   `tensor_tensor_scan` DVE instruction per [128, 300] tile.
4. G_t = U_{t+1} - U_t = DT^2*(goal-y0)*(F_{t+1}-F_t), so the matmul is done
   directly against the differenced basis dPhi (host-precomputed), and the
   a_z*goal term cancels.

Engine budget per core: the two scans per 128-channel chunk are the hard
floor (~43us on the vector engine, ~2.2 cycles/element feedback rate);
everything else (fp32r matmuls on PE, PSUM->SBUF scaled copies on the scalar
engine, batched DMA) is arranged to hide underneath it.
"""

import os
import numpy as np

import concourse.bacc as bacc
import concourse.mybir as mybir
import concourse.tile as tile
from concourse.bass_utils import run_bass_kernel_spmd

# ---- problem constants (hardcoded per contract) ----
B, N, T, DIM, HID = 16384, 32, 300, 2, 64
NCORES = 8
BC = B // NCORES            # 2048 batch rows per core
NJ = BC // 128              # 16 partition blocks per core
NG = 4                      # chunks per output DMA group
DT = float(np.float32(1.0) / np.float32(300.0))
ZS0 = float(np.float32(0.01) * np.float32(DT))   # zs_0 = DT * z0
F32 = mybir.dt.float32
F32R = mybir.dt.float32r
AF = mybir.ActivationFunctionType
AL = mybir.AluOpType

# canvas_r ([64, 372]): wA cols 0:64, oB cols 64:72 (7 used + pad),
# phiX cols 72:372
CR_WA, CR_OB, CR_PHI, CR_W = 0, 64, 72, 372
# canvas_f (fp32, [128, 75]): f1wT cols 0:64, f1b col 64, wAb cols 65:67,
# misc cols 67:75
CF_F1W, CF_F1B, CF_WAB, CF_MISC, CF_W = 0, 64, 65, 67, 75

_CACHE = {}


def _install_ntff_hook():
    """antenv.axon_hooks is missing in this image; shim it so trace=True works."""
    import sys
    import types

    try:
        from antenv.axon_hooks import get_axon_ntff_profile_hook  # noqa: F401
        return
    except ImportError:
        pass
    try:
        import antenv
        from trn_agent_boot.trn_boot import _ntff_profile_via_ctypes
    except ImportError:
        return
    mod = types.ModuleType("antenv.axon_hooks")
    _h = _ntff_profile_via_ctypes("/opt/axon/libaxon_pjrt.so")
    mod.get_axon_ntff_profile_hook = lambda: _h
    mod.set_axon_ntff_profile_hook = lambda h: None
    sys.modules["antenv.axon_hooks"] = mod
    antenv.axon_hooks = mod


def _build():
    nc = bacc.Bacc("TRN2", target_bir_lowering=False, debug=False)

    d_stateT = nc.dram_tensor("stateT", [3, BC], F32, kind="ExternalInput")
    d_y0 = nc.dram_tensor("y0cols", [128, 2 * NJ], F32, kind="ExternalInput")
    d_cr = nc.dram_tensor("canvas_r", [64, CR_W], F32, kind="ExternalInput")
    d_cf = nc.dram_tensor("canvas_f", [128, CF_W], F32, kind="ExternalInput")

    d_z = nc.dram_tensor("z", [2, NJ // NG, 128, NG * T], F32, kind="ExternalOutput")
    d_sig = nc.dram_tensor("sig", [128, 2 * NJ], F32, kind="ExternalOutput")
    d_val = nc.dram_tensor("val", [128, 2 * NJ], F32, kind="ExternalOutput")

    with tile.TileContext(nc) as tc:
        with (
            tc.tile_pool(name="const", bufs=1) as cpool,
            tc.tile_pool(name="psA", bufs=2, space="PSUM") as psA,
            tc.tile_pool(name="psB", bufs=1, space="PSUM") as psB,
            tc.tile_pool(name="psF", bufs=3, space="PSUM") as psF,
            tc.tile_pool(name="work", bufs=3) as wpool,
            tc.tile_pool(name="zout", bufs=3) as zpool,
        ):
            # ---------- input loads (4 DMAs; stateT first: phase 1 needs it) ----------
            stateT = cpool.tile([3, BC], F32)
            cf = cpool.tile([128, CF_W], F32)
            cr = cpool.tile([64, CR_W], F32)
            y0_sb = cpool.tile([128, 2 * NJ], F32)
            nc.sync.dma_start(stateT[:], d_stateT.ap())
            nc.sync.dma_start(cf[:], d_cf.ap())
            nc.gpsimd.dma_start(cr[:], d_cr.ap())
            nc.gpsimd.dma_start(y0_sb[:], d_y0.ap())

            f1wT = cf[0:3, CF_F1W : CF_F1W + HID]
            f1b = cf[0:HID, CF_F1B : CF_F1B + 1]
            wAb = cf[0:N, CF_WAB : CF_WAB + 2]
            misc = cf[:, CF_MISC : CF_MISC + 8]
            wA = cr[:, CR_WA : CR_WA + 2 * N]
            oB = cr[:, CR_OB : CR_OB + 8]
            phiX = cr[0:N, CR_PHI : CR_PHI + T]

            # ---------- phases 1-3 interleaved per 512-wide q-block ----------
            # phase 1: xT = tanh(fc1_w @ state^T + b)       [64, 2048]
            # phase 2: wT_d = (0.1*fc2m_w[w rows_d]) @ x^T + b  (2x [32, 2048])
            # phase 3: orientation B heads, batch on partitions;
            #          cols per block j: [g0, g1, az_raw, sig0, sig1, val0, val1, pad]
            xT = cpool.tile([HID, BC], F32)
            wT = [
                cpool.tile([N, BC], F32, tag=f"wT{d}", name=f"wT{d}")
                for d in range(2)
            ]
            pB = psB.tile([128, 8 * NJ], F32)
            gaz = cpool.tile([128, 8 * NJ], F32)
            for q in range(BC // 512):
                s = slice(512 * q, 512 * (q + 1))
                pX = psA.tile([HID, 512], F32, tag="pX")
                nc.tensor.matmul(pX[:], f1wT, stateT[:, s])
                nc.scalar.activation(xT[:, s], pX[:], AF.Tanh, bias=f1b)
                for d in range(2):
                    pA = psA.tile([N, 512], F32, tag="pA")
                    nc.tensor.matmul(pA[:], wA[:, N * d : N * (d + 1)], xT[:, s])
                    nc.scalar.activation(
                        wT[d][:, s], pA[:], AF.Identity, bias=wAb[:, d : d + 1]
                    )
                for j in range(4 * q, 4 * q + 4):
                    nc.tensor.matmul(
                        pB[:, 8 * j : 8 * j + 8],
                        xT[:, 128 * j : 128 * (j + 1)],
                        oB,
                    )
                nc.scalar.activation(
                    gaz[:, 32 * q : 32 * (q + 1)], pB[:, 32 * q : 32 * (q + 1)],
                    AF.Copy,
                )
            gz = gaz[:].rearrange("p (j i) -> p i j", i=8)  # [128, 8, NJ]

            # ---------- phase 5: per-channel scan constants ----------
            az_t = cpool.tile([128, NJ], F32)
            lam_t = cpool.tile([128, NJ], F32)
            cq_t = cpool.tile([128, NJ], F32)
            s_t = cpool.tile([128, NJ], F32)
            tmp_t = cpool.tile([128, NJ], F32)
            cgy_t = cpool.tile([128, 2 * NJ], F32)
            e0b_t = cpool.tile([128, 2 * NJ], F32)

            # az = clip(az_raw + b_az, 0.5, 30)
            nc.vector.tensor_scalar(az_t[:], gz[:, 2, :], misc[:, 2:3], 0.5, AL.add, AL.max)
            nc.vector.tensor_scalar_min(az_t[:], az_t[:], 30.0)
            # lam = 1 - DT/2 * az
            nc.vector.tensor_scalar(lam_t[:], az_t[:], -DT / 2.0, 1.0, AL.mult, AL.add)
            # cq = az^2 / 4    (= abz)
            nc.vector.tensor_tensor(cq_t[:], az_t[:], az_t[:], AL.mult)
            nc.vector.tensor_scalar_mul(cq_t[:], cq_t[:], 0.25)
            # s = (lam - 1) * zs0 = -DT*zs0/2 * az
            nc.vector.tensor_scalar_mul(s_t[:], az_t[:], -DT * ZS0 / 2.0)
            for d in range(2):
                s = slice(NJ * d, NJ * (d + 1))
                # cgy = DT^2 * (g_raw + b_g - y0)
                nc.vector.tensor_tensor(cgy_t[:, s], gz[:, d, :], y0_sb[:, s], AL.subtract)
                nc.vector.tensor_scalar(
                    cgy_t[:, s], cgy_t[:, s], misc[:, d : d + 1], DT * DT, AL.add, AL.mult
                )
                # e0b = (lam-1)*zs0 + abz*cgy
                nc.vector.tensor_tensor(tmp_t[:], cq_t[:], cgy_t[:, s], AL.mult)
                nc.vector.tensor_tensor(e0b_t[:, s], tmp_t[:], s_t[:], AL.add)

            # ---------- phase 6: 32 chunks of [128 channels x 300 steps] ----------
            for d in range(2):
                for jg in range(NJ // NG):
                    Zb = zpool.tile([128, NG * T], F32, tag="Zb")
                    for js in range(NG):
                        j = jg * NG + js
                        k = NJ * d + j
                        zcol = js * T
                        pF = psF.tile([128, T], F32)
                        nc.tensor.matmul(
                            pF[:], wT[d][:, 128 * j : 128 * (j + 1)], phiX
                        )
                        # G = cgy * [F_0 | dF_0..dF_297]; col0 also gets +e0b -> e_0
                        Gb = wpool.tile([128, T - 1], F32, tag="G")
                        nc.scalar.activation(
                            Gb[:, 0:1], pF[:, 0:1], AF.Identity,
                            bias=e0b_t[:, k : k + 1], scale=cgy_t[:, k : k + 1],
                        )
                        nc.scalar.activation(
                            Gb[:, 1 : T - 1], pF[:, 1 : T - 1], AF.Copy,
                            bias=0.0, scale=cgy_t[:, k : k + 1],
                        )
                        # e-scan: e_i = lam*e_{i-1} + G_{i-1}, e_0 from col0
                        E = wpool.tile([128, T - 1], F32, tag="E")
                        nc.vector.tensor_tensor_scan(
                            E[:],
                            lam_t[:, j : j + 1].broadcast_to([128, T - 1]),
                            Gb[:],
                            0.0,
                            AL.mult,
                            AL.add,
                        )
                        # zs-scan: zs_t = lam*zs_{t-1} + e_{t-1}, zs_0 = ZS0
                        nc.gpsimd.memset(Zb[:, zcol : zcol + 1], ZS0)
                        nc.vector.tensor_tensor_scan(
                            Zb[:, zcol + 1 : zcol + T],
                            lam_t[:, j : j + 1].broadcast_to([128, T - 1]),
                            E[:],
                            ZS0,
                            AL.mult,
                            AL.add,
                        )
                    nc.sync.dma_start(d_z.ap()[d, jg], Zb[:])

            # ---------- sigma / value outputs (off the critical path) ----------
            sig_s = wpool.tile([128, 2 * NJ], F32, tag="sv")
            val_s = wpool.tile([128, 2 * NJ], F32, tag="sv2")
            for d in range(2):
                s = slice(NJ * d, NJ * (d + 1))
                nc.scalar.activation(
                    sig_s[:, s], gz[:, 3 + d, :], AF.Sigmoid,
                    bias=misc[:, 3 + d : 4 + d],
                )
                nc.scalar.activation(
                    val_s[:, s], gz[:, 5 + d, :], AF.Identity,
                    bias=misc[:, 5 + d : 6 + d],
                )
            sig_o = wpool.tile([128, 2 * NJ], F32, tag="sv3")
            nc.vector.tensor_scalar_add(sig_o[:], sig_s[:], 0.001)
            nc.sync.dma_start(d_sig.ap(), sig_o[:])
            nc.sync.dma_start(d_val.ap(), val_s[:])

    nc.compile()
    return nc


def _phi_matrix(dmp_c, dmp_sigma2):
    """Host-precompute [N, T] matrix: col0 = Phi(0), col j = Phi(j)-Phi(j-1)
    for j=1..T-2, col T-1 = 0 (pad for even matmul moving dim).

    Phi(t)[n] = psi_t[n] * x_{t+1} / sum_n psi_t[n], x evolved in fp32 exactly
    as the reference does.
    """
    c = dmp_c.astype(np.float64)
    s2 = dmp_sigma2.astype(np.float64)
    xs = np.empty(T, np.float64)
    x = np.float32(1.0)
    dt32 = np.float32(DT)
    for t in range(T):
        x = np.float32(x + np.float32(np.float32(-x) * dt32))
        xs[t] = np.float64(x)
    psi = np.exp(-0.5 * (xs[:, None] - c[None, :]) ** 2 / s2[None, :])  # [T, N]
    phi = psi * xs[:, None] / psi.sum(axis=1, keepdims=True)            # [T, N]
    out = np.zeros((N, T), np.float64)
    out[:, 0] = phi[0]
    out[:, 1 : T - 1] = (phi[1 : T - 1] - phi[0 : T - 2]).T
    return out.astype(np.float32)


def kernel(state, fc1_w, fc1_b, fc2m_w, fc2m_b, sig_w, sig_b, val_w, val_b,
           dmp_c, dmp_sigma2):
    state = np.asarray(state, np.float32)
    fc1_w = np.asarray(fc1_w, np.float32)
    fc1_b = np.asarray(fc1_b, np.float32)
    fc2m_w = np.asarray(fc2m_w, np.float32)
    fc2m_b = np.asarray(fc2m_b, np.float32)
    sig_w = np.asarray(sig_w, np.float32)
    sig_b = np.asarray(sig_b, np.float32)
    val_w = np.asarray(val_w, np.float32)
    val_b = np.asarray(val_b, np.float32)
    dmp_c = np.asarray(dmp_c, np.float32)
    dmp_sigma2 = np.asarray(dmp_sigma2, np.float32)

    # ---- replicated host-prepped canvases ----
    canvas_r = np.zeros((64, CR_W), np.float32)
    canvas_r[:, CR_WA : CR_WA + 2 * N] = (0.1 * fc2m_w[2 : 2 + 2 * N]).T
    canvas_r[:, CR_OB : CR_OB + 7] = np.stack(
        [
            0.1 * fc2m_w[0], 0.1 * fc2m_w[1], 0.1 * fc2m_w[2 * N + 1],
            0.1 * sig_w[0], 0.1 * sig_w[1],
            0.1 * val_w[0], 0.1 * val_w[1],
        ],
        axis=1,
    )
    canvas_r[0:N, CR_PHI : CR_PHI + T] = _phi_matrix(dmp_c, dmp_sigma2)

    canvas_f = np.zeros((128, CF_W), np.float32)
    canvas_f[0:3, CF_F1W : CF_F1W + HID] = fc1_w.T
    canvas_f[0:HID, CF_F1B] = fc1_b
    canvas_f[0:N, CF_WAB : CF_WAB + 2] = fc2m_b[2 : 2 + 2 * N].reshape(2, N).T
    canvas_f[:, CF_MISC : CF_MISC + 8] = np.array(
        [fc2m_b[0], fc2m_b[1], fc2m_b[2 * N + 1],
         sig_b[0], sig_b[1], val_b[0], val_b[1], 0.0],
        np.float32,
    )

    in_maps = []
    for c in range(NCORES):
        sl = slice(c * BC, (c + 1) * BC)
        sc = state[sl]
        in_maps.append(
            {
                "stateT": np.ascontiguousarray(sc.T),
                "y0cols": np.ascontiguousarray(
                    sc[:, :2].reshape(NJ, 128, 2).transpose(1, 2, 0).reshape(128, 2 * NJ)
                ),
                "canvas_r": canvas_r,
                "canvas_f": canvas_f,
            }
        )

    if "nc" not in _CACHE:
        _CACHE["nc"] = _build()
    nc = _CACHE["nc"]

    trace = bool(int(os.environ.get("CLAUDE_KERNEL_TRACE", "0")))
    if trace:
        _install_ntff_hook()
    res = run_bass_kernel_spmd(
        nc, in_maps, core_ids=list(range(NCORES)), trace=trace
    )
    _CACHE["exec_time_ns"] = res.exec_time_ns
    _CACHE["res"] = res

    # ---- gather / unshard ----
    a = np.empty((B, 2, T), np.float32)
    sig = np.empty((B, 2, 1), np.float32)
    val = np.empty((B, 2), np.float32)
    for c in range(NCORES):
        r = res.results[c]
        sl = slice(c * BC, (c + 1) * BC)
        # z: [2, NJ//NG, 128, NG*T] -> (d, jg, p, js, t) -> (jg, js, p, d, t)
        z = r["z"].reshape(2, NJ // NG, 128, NG, T)
        a[sl] = z.transpose(1, 3, 2, 0, 4).reshape(BC, 2, T)
        sig[sl, :, 0] = (
            r["sig"].reshape(128, 2, NJ).transpose(2, 0, 1).reshape(BC, 2)
        )
        val[sl] = r["val"].reshape(128, 2, NJ).transpose(2, 0, 1).reshape(BC, 2)
    value = np.ascontiguousarray(np.broadcast_to(val[:, :, None], (B, 2, T)))
    return a, sig, value


# revision 13
# speedup vs baseline: 1.0030x; 1.0030x over previous
"""Trainium2 Bass kernel for nn_CNNndpPolicy (CNN-NDP policy head + DMP rollout).

Strategy
--------
Pure data parallel over batch: each of the 8 NeuronCores processes B/8 = 2048
batch rows (M/8 = 4096 DMP channels).

The expensive part of the reference is a 300-step Euler rollout of the DMP
transformation system. Key algebraic facts exploited here:

1. The phase x_t = (1-DT)^t is input-independent, so the RBF basis
   Phi_t = psi(x_t) * x_t / sum(psi(x_t)) is a [300, 32] constant computed on
   the host. The forcing term for all steps is ONE matmul F = w @ Phi^T.
2. The output `a` is a first difference of Y, and y_{t+1} - y_t = DT*z_t,
   so a[:, t] = zs_t with zs = DT*z. Y itself is never materialized.
3. Eliminating y yields a second-order linear recurrence in zs:
       zs_{t+2} = 2*lam*zs_{t+1} - lam^2*zs_t + (U_{t+1} - U_t)
   with lam = 1 - DT*a_z/2 (critically damped: b_z = a_z/4 makes the
   characteristic discriminant exactly zero). This factors into TWO
   first-order scans (e-scan, zs-scan), each a single native
   `tensor_tensor_scan` DVE instruction per [128, 300] tile.
4. G_t = U_{t+1} - U_t = DT^2*(goal-y0)*(F_{t+1}-F_t), so the matmul is done
   directly against the differenced basis dPhi (host-precomputed), and the
   a_z*goal term cancels.

Engine budget per core: the two scans per 128-channel chunk are the hard
floor (~43us on the vector engine, ~2.2 cycles/element feedback rate);
everything else (fp32r matmuls on PE, PSUM->SBUF scaled copies on the scalar
engine, batched DMA) is arranged to hide underneath it.
"""

import os
import numpy as np

import concourse.bacc as bacc
import concourse.mybir as mybir
import concourse.tile as tile
from concourse.bass_utils import run_bass_kernel_spmd

# ---- problem constants (hardcoded per contract) ----
B, N, T, DIM, HID = 16384, 32, 300, 2, 64
NCORES = 8
BC = B // NCORES            # 2048 batch rows per core
NJ = BC // 128              # 16 partition blocks per core
NG = 4                      # chunks per output DMA group
DT = float(np.float32(1.0) / np.float32(300.0))
ZS0 = float(np.float32(0.01) * np.float32(DT))   # zs_0 = DT * z0
F32 = mybir.dt.float32
F32R = mybir.dt.float32r
AF = mybir.ActivationFunctionType
AL = mybir.AluOpType

# canvas_r ([64, 72]): wA cols 0:64, oB cols 64:72 (7 used + pad)
CR_WA, CR_OB, CR_W = 0, 64, 72
# canvas_f (fp32, [128, 75]): f1wT cols 0:64, f1b col 64, wAb cols 65:67,
# misc cols 67:75
CF_F1W, CF_F1B, CF_WAB, CF_MISC, CF_W = 0, 64, 65, 67, 75

_CACHE = {}


def _install_ntff_hook():
    """antenv.axon_hooks is missing in this image; shim it so trace=True works."""
    import sys
    import types

    try:
        from antenv.axon_hooks import get_axon_ntff_profile_hook  # noqa: F401
        return
    except ImportError:
        pass
    try:
        import antenv
        from trn_agent_boot.trn_boot import _ntff_profile_via_ctypes
    except ImportError:
        return
    mod = types.ModuleType("antenv.axon_hooks")
    _h = _ntff_profile_via_ctypes("/opt/axon/libaxon_pjrt.so")
    mod.get_axon_ntff_profile_hook = lambda: _h
    mod.set_axon_ntff_profile_hook = lambda h: None
    sys.modules["antenv.axon_hooks"] = mod
    antenv.axon_hooks = mod


def _build():
    nc = bacc.Bacc("TRN2", target_bir_lowering=False, debug=False)

    d_stateT = nc.dram_tensor("stateT", [3, BC], F32, kind="ExternalInput")
    d_y0 = nc.dram_tensor("y0cols", [128, 2 * NJ], F32, kind="ExternalInput")
    d_cr = nc.dram_tensor("canvas_r", [64, CR_W], F32, kind="ExternalInput")
    d_phiX = nc.dram_tensor("phiX", [N, T], F32R, kind="ExternalInput")
    d_cf = nc.dram_tensor("canvas_f", [128, CF_W], F32, kind="ExternalInput")

    d_z = nc.dram_tensor("z", [2, NJ // NG, 128, NG * T], F32, kind="ExternalOutput")
    d_sig = nc.dram_tensor("sig", [128, 2 * NJ], F32, kind="ExternalOutput")
    d_val = nc.dram_tensor("val", [128, 2 * NJ], F32, kind="ExternalOutput")

    with tile.TileContext(nc) as tc:
        with (
            tc.tile_pool(name="const", bufs=1) as cpool,
            tc.tile_pool(name="psA", bufs=2, space="PSUM") as psA,
            tc.tile_pool(name="psB", bufs=1, space="PSUM") as psB,
            tc.tile_pool(name="psF", bufs=3, space="PSUM") as psF,
            tc.tile_pool(name="work", bufs=3) as wpool,
            tc.tile_pool(name="zout", bufs=3) as zpool,
        ):
            # ---------- input loads (4 DMAs; stateT first: phase 1 needs it) ----------
            stateT = cpool.tile([3, BC], F32)
            cf = cpool.tile([128, CF_W], F32)
            cr = cpool.tile([64, CR_W], F32)
            y0_sb = cpool.tile([128, 2 * NJ], F32)
            nc.sync.dma_start(stateT[:], d_stateT.ap())
            nc.sync.dma_start(cf[:], d_cf.ap())
            phiX_sb = cpool.tile([N, T], F32R)
            nc.gpsimd.dma_start(cr[:], d_cr.ap())
            nc.gpsimd.dma_start(y0_sb[:], d_y0.ap())
            nc.gpsimd.dma_start(phiX_sb[:], d_phiX.ap())

            f1wT = cf[0:3, CF_F1W : CF_F1W + HID]
            f1b = cf[0:HID, CF_F1B : CF_F1B + 1]
            wAb = cf[0:N, CF_WAB : CF_WAB + 2]
            misc = cf[:, CF_MISC : CF_MISC + 8]
            wA = cr[:, CR_WA : CR_WA + 2 * N]
            oB = cr[:, CR_OB : CR_OB + 8]
            phiX = phiX_sb[:]

            # ---------- phases 1-3 interleaved per 512-wide q-block ----------
            # phase 1: xT = tanh(fc1_w @ state^T + b)       [64, 2048]
            # phase 2: wT_d = (0.1*fc2m_w[w rows_d]) @ x^T + b  (2x [32, 2048])
            # phase 3: orientation B heads, batch on partitions;
            #          cols per block j: [g0, g1, az_raw, sig0, sig1, val0, val1, pad]
            xT = cpool.tile([HID, BC], F32)
            wT = [
                cpool.tile([N, BC], F32R, tag=f"wT{d}", name=f"wT{d}")
                for d in range(2)
            ]
            pB = psB.tile([128, 8 * NJ], F32)
            gaz = cpool.tile([128, 8 * NJ], F32)
            for q in range(BC // 512):
                s = slice(512 * q, 512 * (q + 1))
                pX = psA.tile([HID, 512], F32, tag="pX")
                nc.tensor.matmul(pX[:], f1wT, stateT[:, s])
                nc.scalar.activation(xT[:, s], pX[:], AF.Tanh, bias=f1b)
                for d in range(2):
                    pA = psA.tile([N, 512], F32, tag="pA")
                    nc.tensor.matmul(pA[:], wA[:, N * d : N * (d + 1)], xT[:, s])
                    nc.scalar.activation(
                        wT[d][:, s], pA[:], AF.Identity, bias=wAb[:, d : d + 1]
                    )
                for j in range(4 * q, 4 * q + 4):
                    nc.tensor.matmul(
                        pB[:, 8 * j : 8 * j + 8],
                        xT[:, 128 * j : 128 * (j + 1)],
                        oB,
                    )
                nc.scalar.activation(
                    gaz[:, 32 * q : 32 * (q + 1)], pB[:, 32 * q : 32 * (q + 1)],
                    AF.Copy,
                )
            gz = gaz[:].rearrange("p (j i) -> p i j", i=8)  # [128, 8, NJ]

            # ---------- phase 5: per-channel scan constants ----------
            az_t = cpool.tile([128, NJ], F32)
            lam_t = cpool.tile([128, NJ], F32)
            cq_t = cpool.tile([128, NJ], F32)
            s_t = cpool.tile([128, NJ], F32)
            tmp_t = cpool.tile([128, NJ], F32)
            cgy_t = cpool.tile([128, 2 * NJ], F32)
            e0b_t = cpool.tile([128, 2 * NJ], F32)

            # az = clip(az_raw + b_az, 0.5, 30)
            nc.vector.tensor_scalar(az_t[:], gz[:, 2, :], misc[:, 2:3], 0.5, AL.add, AL.max)
            nc.vector.tensor_scalar_min(az_t[:], az_t[:], 30.0)
            # lam = 1 - DT/2 * az
            nc.vector.tensor_scalar(lam_t[:], az_t[:], -DT / 2.0, 1.0, AL.mult, AL.add)
            # cq = az^2 / 4    (= abz)
            nc.vector.tensor_tensor(cq_t[:], az_t[:], az_t[:], AL.mult)
            nc.vector.tensor_scalar_mul(cq_t[:], cq_t[:], 0.25)
            # s = (lam - 1) * zs0 = -DT*zs0/2 * az
            nc.vector.tensor_scalar_mul(s_t[:], az_t[:], -DT * ZS0 / 2.0)
            for d in range(2):
                s = slice(NJ * d, NJ * (d + 1))
                # cgy = DT^2 * (g_raw + b_g - y0)
                nc.vector.tensor_tensor(cgy_t[:, s], gz[:, d, :], y0_sb[:, s], AL.subtract)
                nc.vector.tensor_scalar(
                    cgy_t[:, s], cgy_t[:, s], misc[:, d : d + 1], DT * DT, AL.add, AL.mult
                )
                # e0b = (lam-1)*zs0 + abz*cgy
                nc.vector.tensor_tensor(tmp_t[:], cq_t[:], cgy_t[:, s], AL.mult)
                nc.vector.tensor_tensor(e0b_t[:, s], tmp_t[:], s_t[:], AL.add)

            # ---------- phase 6: 32 chunks of [128 channels x 300 steps] ----------
            for d in range(2):
                for jg in range(NJ // NG):
                    Zb = zpool.tile([128, NG * T], F32, tag="Zb")
                    for js in range(NG):
                        j = jg * NG + js
                        k = NJ * d + j
                        zcol = js * T
                        pF = psF.tile([128, T], F32)
                        nc.tensor.matmul(
                            pF[:], wT[d][:, 128 * j : 128 * (j + 1)], phiX
                        )
                        # G = cgy * [F_0 | dF_0..dF_297]; col0 also gets +e0b -> e_0
                        Gb = wpool.tile([128, T - 1], F32, tag="G")
                        nc.scalar.activation(
                            Gb[:, 0:1], pF[:, 0:1], AF.Identity,
                            bias=e0b_t[:, k : k + 1], scale=cgy_t[:, k : k + 1],
                        )
                        nc.scalar.activation(
                            Gb[:, 1 : T - 1], pF[:, 1 : T - 1], AF.Copy,
                            bias=0.0, scale=cgy_t[:, k : k + 1],
                        )
                        # e-scan: e_i = lam*e_{i-1} + G_{i-1}, e_0 from col0
                        E = wpool.tile([128, T - 1], F32, tag="E")
                        nc.vector.tensor_tensor_scan(
                            E[:],
                            lam_t[:, j : j + 1].broadcast_to([128, T - 1]),
                            Gb[:],
                            0.0,
                            AL.mult,
                            AL.add,
                        )
                        # zs-scan: zs_t = lam*zs_{t-1} + e_{t-1}, zs_0 = ZS0
                        nc.gpsimd.memset(Zb[:, zcol : zcol + 1], ZS0)
                        nc.vector.tensor_tensor_scan(
                            Zb[:, zcol + 1 : zcol + T],
                            lam_t[:, j : j + 1].broadcast_to([128, T - 1]),
                            E[:],
                            ZS0,
                            AL.mult,
                            AL.add,
                        )
                    nc.sync.dma_start(d_z.ap()[d, jg], Zb[:])

            # ---------- sigma / value outputs (off the critical path) ----------
            sig_s = wpool.tile([128, 2 * NJ], F32, tag="sv")
            val_s = wpool.tile([128, 2 * NJ], F32, tag="sv2")
            for d in range(2):
                s = slice(NJ * d, NJ * (d + 1))
                nc.scalar.activation(
                    sig_s[:, s], gz[:, 3 + d, :], AF.Sigmoid,
                    bias=misc[:, 3 + d : 4 + d],
                )
                nc.scalar.activation(
                    val_s[:, s], gz[:, 5 + d, :], AF.Identity,
                    bias=misc[:, 5 + d : 6 + d],
                )
            sig_o = wpool.tile([128, 2 * NJ], F32, tag="sv3")
            nc.vector.tensor_scalar_add(sig_o[:], sig_s[:], 0.001)
            nc.sync.dma_start(d_sig.ap(), sig_o[:])
            nc.sync.dma_start(d_val.ap(), val_s[:])

    nc.compile()
    return nc


def _phi_matrix(dmp_c, dmp_sigma2):
    """Host-precompute [N, T] matrix: col0 = Phi(0), col j = Phi(j)-Phi(j-1)
    for j=1..T-2, col T-1 = 0 (pad for even matmul moving dim).

    Phi(t)[n] = psi_t[n] * x_{t+1} / sum_n psi_t[n], x evolved in fp32 exactly
    as the reference does.
    """
    c = dmp_c.astype(np.float64)
    s2 = dmp_sigma2.astype(np.float64)
    xs = np.empty(T, np.float64)
    x = np.float32(1.0)
    dt32 = np.float32(DT)
    for t in range(T):
        x = np.float32(x + np.float32(np.float32(-x) * dt32))
        xs[t] = np.float64(x)
    psi = np.exp(-0.5 * (xs[:, None] - c[None, :]) ** 2 / s2[None, :])  # [T, N]
    phi = psi * xs[:, None] / psi.sum(axis=1, keepdims=True)            # [T, N]
    out = np.zeros((N, T), np.float64)
    out[:, 0] = phi[0]
    out[:, 1 : T - 1] = (phi[1 : T - 1] - phi[0 : T - 2]).T
    return out.astype(np.float32)


def kernel(state, fc1_w, fc1_b, fc2m_w, fc2m_b, sig_w, sig_b, val_w, val_b,
           dmp_c, dmp_sigma2):
    state = np.asarray(state, np.float32)
    fc1_w = np.asarray(fc1_w, np.float32)
    fc1_b = np.asarray(fc1_b, np.float32)
    fc2m_w = np.asarray(fc2m_w, np.float32)
    fc2m_b = np.asarray(fc2m_b, np.float32)
    sig_w = np.asarray(sig_w, np.float32)
    sig_b = np.asarray(sig_b, np.float32)
    val_w = np.asarray(val_w, np.float32)
    val_b = np.asarray(val_b, np.float32)
    dmp_c = np.asarray(dmp_c, np.float32)
    dmp_sigma2 = np.asarray(dmp_sigma2, np.float32)

    # ---- replicated host-prepped canvases ----
    canvas_r = np.zeros((64, CR_W), np.float32)
    canvas_r[:, CR_WA : CR_WA + 2 * N] = (0.1 * fc2m_w[2 : 2 + 2 * N]).T
    canvas_r[:, CR_OB : CR_OB + 7] = np.stack(
        [
            0.1 * fc2m_w[0], 0.1 * fc2m_w[1], 0.1 * fc2m_w[2 * N + 1],
            0.1 * sig_w[0], 0.1 * sig_w[1],
            0.1 * val_w[0], 0.1 * val_w[1],
        ],
        axis=1,
    )

    phiX = _phi_matrix(dmp_c, dmp_sigma2)

    canvas_f = np.zeros((128, CF_W), np.float32)
    canvas_f[0:3, CF_F1W : CF_F1W + HID] = fc1_w.T
    canvas_f[0:HID, CF_F1B] = fc1_b
    canvas_f[0:N, CF_WAB : CF_WAB + 2] = fc2m_b[2 : 2 + 2 * N].reshape(2, N).T
    canvas_f[:, CF_MISC : CF_MISC + 8] = np.array(
        [fc2m_b[0], fc2m_b[1], fc2m_b[2 * N + 1],
         sig_b[0], sig_b[1], val_b[0], val_b[1], 0.0],
        np.float32,
    )

    in_maps = []
    for c in range(NCORES):
        sl = slice(c * BC, (c + 1) * BC)
        sc = state[sl]
        in_maps.append(
            {
                "stateT": np.ascontiguousarray(sc.T),
                "y0cols": np.ascontiguousarray(
                    sc[:, :2].reshape(NJ, 128, 2).transpose(1, 2, 0).reshape(128, 2 * NJ)
                ),
                "canvas_r": canvas_r,
                "canvas_f": canvas_f,
                "phiX": phiX,
            }
        )

    if "nc" not in _CACHE:
        _CACHE["nc"] = _build()
    nc = _CACHE["nc"]

    trace = bool(int(os.environ.get("CLAUDE_KERNEL_TRACE", "0")))
    if trace:
        _install_ntff_hook()
    res = run_bass_kernel_spmd(
        nc, in_maps, core_ids=list(range(NCORES)), trace=trace
    )
    _CACHE["exec_time_ns"] = res.exec_time_ns
    _CACHE["res"] = res

    # ---- gather / unshard ----
    a = np.empty((B, 2, T), np.float32)
    sig = np.empty((B, 2, 1), np.float32)
    val = np.empty((B, 2), np.float32)
    for c in range(NCORES):
        r = res.results[c]
        sl = slice(c * BC, (c + 1) * BC)
        # z: [2, NJ//NG, 128, NG*T] -> (d, jg, p, js, t) -> (jg, js, p, d, t)
        z = r["z"].reshape(2, NJ // NG, 128, NG, T)
        a[sl] = z.transpose(1, 3, 2, 0, 4).reshape(BC, 2, T)
        sig[sl, :, 0] = (
            r["sig"].reshape(128, 2, NJ).transpose(2, 0, 1).reshape(BC, 2)
        )
        val[sl] = r["val"].reshape(128, 2, NJ).transpose(2, 0, 1).reshape(BC, 2)
    value = np.ascontiguousarray(np.broadcast_to(val[:, :, None], (B, 2, T)))
    return a, sig, value


# revision 14
# speedup vs baseline: 1.0655x; 1.0624x over previous
"""Trainium2 Bass kernel for nn_CNNndpPolicy (CNN-NDP policy head + DMP rollout).

Strategy
--------
Pure data parallel over batch: each of the 8 NeuronCores processes B/8 = 2048
batch rows (M/8 = 4096 DMP channels).

The expensive part of the reference is a 300-step Euler rollout of the DMP
transformation system. Key algebraic facts exploited here:

1. The phase x_t = (1-DT)^t is input-independent, so the RBF basis
   Phi_t = psi(x_t) * x_t / sum(psi(x_t)) is a [300, 32] constant computed on
   the host. The forcing term for all steps is ONE matmul F = w @ Phi^T.
2. The output `a` is a first difference of Y, and y_{t+1} - y_t = DT*z_t,
   so a[:, t] = zs_t with zs = DT*z. Y itself is never materialized.
3. Eliminating y yields a second-order linear recurrence in zs:
       zs_{t+2} = 2*lam*zs_{t+1} - lam^2*zs_t + (U_{t+1} - U_t)
   with lam = 1 - DT*a_z/2 (critically damped: b_z = a_z/4 makes the
   characteristic discriminant exactly zero). This factors into TWO
   first-order scans (e-scan, zs-scan), each a single native
   `tensor_tensor_scan` DVE instruction per [128, 300] tile.
4. G_t = U_{t+1} - U_t = DT^2*(goal-y0)*(F_{t+1}-F_t), so the matmul is done
   directly against the differenced basis dPhi (host-precomputed), and the
   a_z*goal term cancels.

Engine budget per core: the two scans per 128-channel chunk are the hard
floor (~43us on the vector engine, ~2.2 cycles/element feedback rate);
everything else (fp32r matmuls on PE, PSUM->SBUF scaled copies on the scalar
engine, batched DMA) is arranged to hide underneath it.
"""

import os
import numpy as np

import concourse.bacc as bacc
import concourse.mybir as mybir
import concourse.tile as tile
from concourse.bass_utils import run_bass_kernel_spmd

# ---- problem constants (hardcoded per contract) ----
B, N, T, DIM, HID = 16384, 32, 300, 2, 64
NCORES = 8
BC = B // NCORES            # 2048 batch rows per core
NJ = BC // 128              # 16 partition blocks per core
NG = 4                      # chunks per output DMA group
DT = float(np.float32(1.0) / np.float32(300.0))
ZS0 = float(np.float32(0.01) * np.float32(DT))   # zs_0 = DT * z0
F32 = mybir.dt.float32
F32R = mybir.dt.float32r
AF = mybir.ActivationFunctionType
AL = mybir.AluOpType

# canvas_r ([64, 72]): wA cols 0:64, oB cols 64:72 (7 used + pad)
CR_WA, CR_OB, CR_W = 0, 64, 72
# canvas_f (fp32, [128, 75]): f1wT cols 0:64, f1b col 64, wAb cols 65:67,
# misc cols 67:75
CF_F1W, CF_F1B, CF_WAB, CF_MISC, CF_W = 0, 64, 65, 67, 75

_CACHE = {}


def _install_ntff_hook():
    """antenv.axon_hooks is missing in this image; shim it so trace=True works."""
    import sys
    import types

    try:
        from antenv.axon_hooks import get_axon_ntff_profile_hook  # noqa: F401
        return
    except ImportError:
        pass
    try:
        import antenv
        from trn_agent_boot.trn_boot import _ntff_profile_via_ctypes
    except ImportError:
        return
    mod = types.ModuleType("antenv.axon_hooks")
    _h = _ntff_profile_via_ctypes("/opt/axon/libaxon_pjrt.so")
    mod.get_axon_ntff_profile_hook = lambda: _h
    mod.set_axon_ntff_profile_hook = lambda h: None
    sys.modules["antenv.axon_hooks"] = mod
    antenv.axon_hooks = mod


def _build():
    nc = bacc.Bacc("TRN2", target_bir_lowering=False, debug=False)

    d_stateT = nc.dram_tensor("stateT", [3, BC], F32, kind="ExternalInput")
    d_y0 = nc.dram_tensor("y0cols", [128, 2 * NJ], F32, kind="ExternalInput")
    d_cr = nc.dram_tensor("canvas_r", [64, CR_W], F32, kind="ExternalInput")
    d_phiX = nc.dram_tensor("phiX", [2 * N, T], F32R, kind="ExternalInput")
    d_cf = nc.dram_tensor("canvas_f", [128, CF_W], F32, kind="ExternalInput")

    d_z = nc.dram_tensor("z", [2, NJ // NG, 128, NG * T], F32, kind="ExternalOutput")
    d_sig = nc.dram_tensor("sig", [128, 2 * NJ], F32, kind="ExternalOutput")
    d_val = nc.dram_tensor("val", [128, 2 * NJ], F32, kind="ExternalOutput")

    with tile.TileContext(nc) as tc:
        with (
            tc.tile_pool(name="const", bufs=1) as cpool,
            tc.tile_pool(name="psA", bufs=2, space="PSUM") as psA,
            tc.tile_pool(name="psB", bufs=1, space="PSUM") as psB,
            tc.tile_pool(name="psF", bufs=3, space="PSUM") as psF,
            tc.tile_pool(name="work", bufs=3) as wpool,
            tc.tile_pool(name="zout", bufs=3) as zpool,
        ):
            # ---------- input loads (4 DMAs; stateT first: phase 1 needs it) ----------
            stateT = cpool.tile([3, BC], F32)
            cf = cpool.tile([128, CF_W], F32)
            cr = cpool.tile([64, CR_W], F32)
            y0_sb = cpool.tile([128, 2 * NJ], F32)
            nc.sync.dma_start(stateT[:], d_stateT.ap())
            nc.sync.dma_start(cf[:], d_cf.ap())
            phiX_sb = cpool.tile([2 * N, T], F32R)
            nc.gpsimd.dma_start(cr[:], d_cr.ap())
            nc.gpsimd.dma_start(y0_sb[:], d_y0.ap())
            nc.gpsimd.dma_start(phiX_sb[:], d_phiX.ap())

            f1wT = cf[0:3, CF_F1W : CF_F1W + HID]
            f1b = cf[0:HID, CF_F1B : CF_F1B + 1]
            wAb = cf[0:2 * N, CF_WAB : CF_WAB + 1]
            misc = cf[:, CF_MISC : CF_MISC + 8]
            wA = cr[:, CR_WA : CR_WA + 2 * N]
            oB = cr[:, CR_OB : CR_OB + 8]
            phiX = phiX_sb[:]

            # ---------- phases 1-3, phase-major so each engine's queue streams ----------
            # phase 1: xT = tanh(fc1_w @ state^T + b)         [64, 2048]
            # phase 2: wTall = (0.1*fc2m_w[2:66]) @ x^T + b   [64, 2048] fp32r
            #          (rows 0:32 = d0 weights^T, rows 32:64 = d1; the chunk
            #          matmul pairs wTall[32d:...] with the row-duplicated
            #          phiX at the same base partition)
            # phase 3: orientation B heads, batch on partitions;
            #          cols per block j: [g0, g1, az_raw, sig0, sig1, val0, val1, pad]
            xT = cpool.tile([HID, BC], F32)
            wTall = cpool.tile([2 * N, BC], F32R)
            pB = psB.tile([128, 8 * NJ], F32)
            gaz = cpool.tile([128, 8 * NJ], F32)
            for q in range(BC // 512):
                s = slice(512 * q, 512 * (q + 1))
                pX = psA.tile([HID, 512], F32, tag="pX")
                nc.tensor.matmul(pX[:], f1wT, stateT[:, s])
                nc.scalar.activation(xT[:, s], pX[:], AF.Tanh, bias=f1b)
            for q in range(BC // 512):
                s = slice(512 * q, 512 * (q + 1))
                pA = psA.tile([2 * N, 512], F32, tag="pA")
                nc.tensor.matmul(pA[:], wA, xT[:, s])
                nc.scalar.activation(wTall[:, s], pA[:], AF.Identity, bias=wAb)
            for j in range(NJ):
                nc.tensor.matmul(
                    pB[:, 8 * j : 8 * j + 8],
                    xT[:, 128 * j : 128 * (j + 1)],
                    oB,
                )
                if j % 4 == 3:
                    q = j // 4
                    nc.scalar.activation(
                        gaz[:, 32 * q : 32 * (q + 1)], pB[:, 32 * q : 32 * (q + 1)],
                        AF.Copy,
                    )
            gz = gaz[:].rearrange("p (j i) -> p i j", i=8)  # [128, 8, NJ]

            # ---------- phase 5: per-channel scan constants ----------
            az_t = cpool.tile([128, NJ], F32)
            lam_t = cpool.tile([128, NJ], F32)
            cq_t = cpool.tile([128, NJ], F32)
            s_t = cpool.tile([128, NJ], F32)
            tmp_t = cpool.tile([128, NJ], F32)
            cgy_t = cpool.tile([128, 2 * NJ], F32)
            e0b_t = cpool.tile([128, 2 * NJ], F32)

            # az = clip(az_raw + b_az, 0.5, 30)
            nc.vector.tensor_scalar(az_t[:], gz[:, 2, :], misc[:, 2:3], 0.5, AL.add, AL.max)
            nc.vector.tensor_scalar_min(az_t[:], az_t[:], 30.0)
            # lam = 1 - DT/2 * az
            nc.vector.tensor_scalar(lam_t[:], az_t[:], -DT / 2.0, 1.0, AL.mult, AL.add)
            # cq = az^2 / 4    (= abz)
            nc.vector.tensor_tensor(cq_t[:], az_t[:], az_t[:], AL.mult)
            nc.vector.tensor_scalar_mul(cq_t[:], cq_t[:], 0.25)
            # s = (lam - 1) * zs0 = -DT*zs0/2 * az
            nc.vector.tensor_scalar_mul(s_t[:], az_t[:], -DT * ZS0 / 2.0)
            for d in range(2):
                s = slice(NJ * d, NJ * (d + 1))
                # cgy = DT^2 * (g_raw + b_g - y0)
                nc.vector.tensor_tensor(cgy_t[:, s], gz[:, d, :], y0_sb[:, s], AL.subtract)
                nc.vector.tensor_scalar(
                    cgy_t[:, s], cgy_t[:, s], misc[:, d : d + 1], DT * DT, AL.add, AL.mult
                )
                # e0b = (lam-1)*zs0 + abz*cgy
                nc.vector.tensor_tensor(tmp_t[:], cq_t[:], cgy_t[:, s], AL.mult)
                nc.vector.tensor_tensor(e0b_t[:, s], tmp_t[:], s_t[:], AL.add)

            # ---------- phase 6: 32 chunks of [128 channels x 300 steps] ----------
            for d in range(2):
                for jg in range(NJ // NG):
                    Zb = zpool.tile([128, NG * T], F32, tag="Zb")
                    for js in range(NG):
                        j = jg * NG + js
                        k = NJ * d + j
                        zcol = js * T
                        pF = psF.tile([128, T], F32)
                        nc.tensor.matmul(
                            pF[:],
                            wTall[N * d : N * (d + 1), 128 * j : 128 * (j + 1)],
                            phiX_sb[N * d : N * (d + 1), :],
                        )
                        # G = cgy * [F_0 | dF_0..dF_297]; col0 also gets +e0b -> e_0
                        Gb = wpool.tile([128, T - 1], F32, tag="G")
                        nc.scalar.activation(
                            Gb[:, 0:1], pF[:, 0:1], AF.Identity,
                            bias=e0b_t[:, k : k + 1], scale=cgy_t[:, k : k + 1],
                        )
                        nc.scalar.activation(
                            Gb[:, 1 : T - 1], pF[:, 1 : T - 1], AF.Copy,
                            bias=0.0, scale=cgy_t[:, k : k + 1],
                        )
                        # e-scan: e_i = lam*e_{i-1} + G_{i-1}, e_0 from col0
                        E = wpool.tile([128, T - 1], F32, tag="E")
                        nc.vector.tensor_tensor_scan(
                            E[:],
                            lam_t[:, j : j + 1].broadcast_to([128, T - 1]),
                            Gb[:],
                            0.0,
                            AL.mult,
                            AL.add,
                        )
                        # zs-scan: zs_t = lam*zs_{t-1} + e_{t-1}, zs_0 = ZS0
                        nc.gpsimd.memset(Zb[:, zcol : zcol + 1], ZS0)
                        nc.vector.tensor_tensor_scan(
                            Zb[:, zcol + 1 : zcol + T],
                            lam_t[:, j : j + 1].broadcast_to([128, T - 1]),
                            E[:],
                            ZS0,
                            AL.mult,
                            AL.add,
                        )
                    nc.sync.dma_start(d_z.ap()[d, jg], Zb[:])

            # ---------- sigma / value outputs (off the critical path) ----------
            sig_s = wpool.tile([128, 2 * NJ], F32, tag="sv")
            val_s = wpool.tile([128, 2 * NJ], F32, tag="sv2")
            for d in range(2):
                s = slice(NJ * d, NJ * (d + 1))
                nc.scalar.activation(
                    sig_s[:, s], gz[:, 3 + d, :], AF.Sigmoid,
                    bias=misc[:, 3 + d : 4 + d],
                )
                nc.scalar.activation(
                    val_s[:, s], gz[:, 5 + d, :], AF.Identity,
                    bias=misc[:, 5 + d : 6 + d],
                )
            sig_o = wpool.tile([128, 2 * NJ], F32, tag="sv3")
            nc.vector.tensor_scalar_add(sig_o[:], sig_s[:], 0.001)
            nc.sync.dma_start(d_sig.ap(), sig_o[:])
            nc.sync.dma_start(d_val.ap(), val_s[:])

    nc.compile()
    return nc


def _phi_matrix(dmp_c, dmp_sigma2):
    """Host-precompute [N, T] matrix: col0 = Phi(0), col j = Phi(j)-Phi(j-1)
    for j=1..T-2, col T-1 = 0 (pad for even matmul moving dim).

    Phi(t)[n] = psi_t[n] * x_{t+1} / sum_n psi_t[n], x evolved in fp32 exactly
    as the reference does.
    """
    c = dmp_c.astype(np.float64)
    s2 = dmp_sigma2.astype(np.float64)
    xs = np.empty(T, np.float64)
    x = np.float32(1.0)
    dt32 = np.float32(DT)
    for t in range(T):
        x = np.float32(x + np.float32(np.float32(-x) * dt32))
        xs[t] = np.float64(x)
    psi = np.exp(-0.5 * (xs[:, None] - c[None, :]) ** 2 / s2[None, :])  # [T, N]
    phi = psi * xs[:, None] / psi.sum(axis=1, keepdims=True)            # [T, N]
    out = np.zeros((N, T), np.float64)
    out[:, 0] = phi[0]
    out[:, 1 : T - 1] = (phi[1 : T - 1] - phi[0 : T - 2]).T
    return out.astype(np.float32)


def kernel(state, fc1_w, fc1_b, fc2m_w, fc2m_b, sig_w, sig_b, val_w, val_b,
           dmp_c, dmp_sigma2):
    state = np.asarray(state, np.float32)
    fc1_w = np.asarray(fc1_w, np.float32)
    fc1_b = np.asarray(fc1_b, np.float32)
    fc2m_w = np.asarray(fc2m_w, np.float32)
    fc2m_b = np.asarray(fc2m_b, np.float32)
    sig_w = np.asarray(sig_w, np.float32)
    sig_b = np.asarray(sig_b, np.float32)
    val_w = np.asarray(val_w, np.float32)
    val_b = np.asarray(val_b, np.float32)
    dmp_c = np.asarray(dmp_c, np.float32)
    dmp_sigma2 = np.asarray(dmp_sigma2, np.float32)

    # ---- replicated host-prepped canvases ----
    canvas_r = np.zeros((64, CR_W), np.float32)
    canvas_r[:, CR_WA : CR_WA + 2 * N] = (0.1 * fc2m_w[2 : 2 + 2 * N]).T
    canvas_r[:, CR_OB : CR_OB + 7] = np.stack(
        [
            0.1 * fc2m_w[0], 0.1 * fc2m_w[1], 0.1 * fc2m_w[2 * N + 1],
            0.1 * sig_w[0], 0.1 * sig_w[1],
            0.1 * val_w[0], 0.1 * val_w[1],
        ],
        axis=1,
    )

    phiX = np.ascontiguousarray(np.tile(_phi_matrix(dmp_c, dmp_sigma2), (2, 1)))

    canvas_f = np.zeros((128, CF_W), np.float32)
    canvas_f[0:3, CF_F1W : CF_F1W + HID] = fc1_w.T
    canvas_f[0:HID, CF_F1B] = fc1_b
    canvas_f[0 : 2 * N, CF_WAB] = fc2m_b[2 : 2 + 2 * N]
    canvas_f[:, CF_MISC : CF_MISC + 8] = np.array(
        [fc2m_b[0], fc2m_b[1], fc2m_b[2 * N + 1],
         sig_b[0], sig_b[1], val_b[0], val_b[1], 0.0],
        np.float32,
    )

    in_maps = []
    for c in range(NCORES):
        sl = slice(c * BC, (c + 1) * BC)
        sc = state[sl]
        in_maps.append(
            {
                "stateT": np.ascontiguousarray(sc.T),
                "y0cols": np.ascontiguousarray(
                    sc[:, :2].reshape(NJ, 128, 2).transpose(1, 2, 0).reshape(128, 2 * NJ)
                ),
                "canvas_r": canvas_r,
                "canvas_f": canvas_f,
                "phiX": phiX,
            }
        )

    if "nc" not in _CACHE:
        _CACHE["nc"] = _build()
    nc = _CACHE["nc"]

    trace = bool(int(os.environ.get("CLAUDE_KERNEL_TRACE", "0")))
    if trace:
        _install_ntff_hook()
    res = run_bass_kernel_spmd(
        nc, in_maps, core_ids=list(range(NCORES)), trace=trace
    )
    _CACHE["exec_time_ns"] = res.exec_time_ns
    _CACHE["res"] = res

    # ---- gather / unshard ----
    a = np.empty((B, 2, T), np.float32)
    sig = np.empty((B, 2, 1), np.float32)
    val = np.empty((B, 2), np.float32)
    for c in range(NCORES):
        r = res.results[c]
        sl = slice(c * BC, (c + 1) * BC)
        # z: [2, NJ//NG, 128, NG*T] -> (d, jg, p, js, t) -> (jg, js, p, d, t)
        z = r["z"].reshape(2, NJ // NG, 128, NG, T)
        a[sl] = z.transpose(1, 3, 2, 0, 4).reshape(BC, 2, T)
        sig[sl, :, 0] = (
            r["sig"].reshape(128, 2, NJ).transpose(2, 0, 1).reshape(BC, 2)
        )
        val[sl] = r["val"].reshape(128, 2, NJ).transpose(2, 0, 1).reshape(BC, 2)
    value = np.ascontiguousarray(np.broadcast_to(val[:, :, None], (B, 2, T)))
    return a, sig, value


# revision 15
# speedup vs baseline: 1.2150x; 1.1403x over previous
"""Trainium2 Bass kernel for nn_CNNndpPolicy (CNN-NDP policy head + DMP rollout).

Strategy
--------
Pure data parallel over batch: each of the 8 NeuronCores processes B/8 = 2048
batch rows (M/8 = 4096 DMP channels).

The expensive part of the reference is a 300-step Euler rollout of the DMP
transformation system. Key algebraic facts exploited here:

1. The phase x_t = (1-DT)^t is input-independent, so the RBF basis
   Phi_t = psi(x_t) * x_t / sum(psi(x_t)) is a [300, 32] constant computed on
   the host. The forcing term for all steps is ONE matmul F = w @ Phi^T.
2. The output `a` is a first difference of Y, and y_{t+1} - y_t = DT*z_t,
   so a[:, t] = zs_t with zs = DT*z. Y itself is never materialized.
3. Eliminating y yields a second-order linear recurrence in zs:
       zs_{t+2} = 2*lam*zs_{t+1} - lam^2*zs_t + (U_{t+1} - U_t)
   with lam = 1 - DT*a_z/2 (critically damped: b_z = a_z/4 makes the
   characteristic discriminant exactly zero). This factors into TWO
   first-order scans (e-scan, zs-scan), each a single native
   `tensor_tensor_scan` DVE instruction per [128, 300] tile.
4. G_t = U_{t+1} - U_t = DT^2*(goal-y0)*(F_{t+1}-F_t), so the matmul is done
   directly against the differenced basis dPhi (host-precomputed), and the
   a_z*goal term cancels.

Engine budget per core: the two scans per 128-channel chunk are the hard
floor (~43us on the vector engine, ~2.2 cycles/element feedback rate);
everything else (fp32r matmuls on PE, PSUM->SBUF scaled copies on the scalar
engine, batched DMA) is arranged to hide underneath it.
"""

import os
import numpy as np

import concourse.bacc as bacc
import concourse.mybir as mybir
import concourse.tile as tile
from concourse.bass_utils import run_bass_kernel_spmd

# ---- problem constants (hardcoded per contract) ----
B, N, T, DIM, HID = 16384, 32, 300, 2, 64
NCORES = 8
BC = B // NCORES            # 2048 batch rows per core
NJ = BC // 128              # 16 partition blocks per core
NG = 4                      # chunks per output DMA group
DT = float(np.float32(1.0) / np.float32(300.0))
ZS0 = float(np.float32(0.01) * np.float32(DT))   # zs_0 = DT * z0
F32 = mybir.dt.float32
F32R = mybir.dt.float32r
AF = mybir.ActivationFunctionType
AL = mybir.AluOpType

# canvas_r (fp32r, [64, 136]): wA cols 0:64, oB cols 64:72 (7 used + pad),
# f1wT cols 72:136 (rows 0:3)
CR_WA, CR_OB, CR_F1W, CR_W = 0, 64, 72, 136
# canvas_f (fp32, [128, 11]): f1b col 0, wAb col 1, misc cols 3:11
CF_F1B, CF_WAB, CF_MISC, CF_W = 0, 1, 3, 11

_CACHE = {}


def _install_ntff_hook():
    """antenv.axon_hooks is missing in this image; shim it so trace=True works."""
    import sys
    import types

    try:
        from antenv.axon_hooks import get_axon_ntff_profile_hook  # noqa: F401
        return
    except ImportError:
        pass
    try:
        import antenv
        from trn_agent_boot.trn_boot import _ntff_profile_via_ctypes
    except ImportError:
        return
    mod = types.ModuleType("antenv.axon_hooks")
    _h = _ntff_profile_via_ctypes("/opt/axon/libaxon_pjrt.so")
    mod.get_axon_ntff_profile_hook = lambda: _h
    mod.set_axon_ntff_profile_hook = lambda h: None
    sys.modules["antenv.axon_hooks"] = mod
    antenv.axon_hooks = mod


def _build():
    nc = bacc.Bacc("TRN2", target_bir_lowering=False, debug=False)

    d_stateT = nc.dram_tensor("stateT", [3, BC], F32R, kind="ExternalInput")
    d_y0 = nc.dram_tensor("y0cols", [128, 2 * NJ], F32, kind="ExternalInput")
    d_cr = nc.dram_tensor("canvas_r", [64, CR_W], F32R, kind="ExternalInput")
    d_phiX = nc.dram_tensor("phiX", [2 * N, T], F32R, kind="ExternalInput")
    d_cf = nc.dram_tensor("canvas_f", [128, CF_W], F32, kind="ExternalInput")

    d_z = nc.dram_tensor("z", [2, NJ // NG, 128, NG * T], F32, kind="ExternalOutput")
    d_sig = nc.dram_tensor("sig", [128, 2 * NJ], F32, kind="ExternalOutput")
    d_val = nc.dram_tensor("val", [128, 2 * NJ], F32, kind="ExternalOutput")

    with tile.TileContext(nc) as tc:
        with (
            tc.tile_pool(name="const", bufs=1) as cpool,
            tc.tile_pool(name="psA", bufs=2, space="PSUM") as psA,
            tc.tile_pool(name="psB", bufs=1, space="PSUM") as psB,
            tc.tile_pool(name="psF", bufs=3, space="PSUM") as psF,
            tc.tile_pool(name="work", bufs=3) as wpool,
            tc.tile_pool(name="zout", bufs=3) as zpool,
        ):
            # ---------- input loads (4 DMAs; stateT first: phase 1 needs it) ----------
            stateT = cpool.tile([3, BC], F32R)
            cf = cpool.tile([128, CF_W], F32)
            cr = cpool.tile([64, CR_W], F32R)
            y0_sb = cpool.tile([128, 2 * NJ], F32)
            nc.sync.dma_start(stateT[:], d_stateT.ap())
            nc.sync.dma_start(cf[:], d_cf.ap())
            phiX_sb = cpool.tile([2 * N, T], F32R)
            nc.gpsimd.dma_start(cr[:], d_cr.ap())
            nc.gpsimd.dma_start(y0_sb[:], d_y0.ap())
            nc.gpsimd.dma_start(phiX_sb[:], d_phiX.ap())

            f1b = cf[0:HID, CF_F1B : CF_F1B + 1]
            wAb = cf[0:2 * N, CF_WAB : CF_WAB + 1]
            misc = cf[:, CF_MISC : CF_MISC + 8]
            f1wT = cr[0:3, CR_F1W : CR_F1W + HID]
            wA = cr[:, CR_WA : CR_WA + 2 * N]
            oB = cr[:, CR_OB : CR_OB + 8]
            phiX = phiX_sb[:]

            # ---------- phases 1-3, phase-major so each engine's queue streams ----------
            # phase 1: xT = tanh(fc1_w @ state^T + b)         [64, 2048]
            # phase 2: wTall = (0.1*fc2m_w[2:66]) @ x^T + b   [64, 2048] fp32r
            #          (rows 0:32 = d0 weights^T, rows 32:64 = d1; the chunk
            #          matmul pairs wTall[32d:...] with the row-duplicated
            #          phiX at the same base partition)
            # phase 3: orientation B heads, batch on partitions;
            #          cols per block j: [g0, g1, az_raw, sig0, sig1, val0, val1, pad]
            xT = cpool.tile([HID, BC], F32R)
            wTall = cpool.tile([2 * N, BC], F32R)
            pB = psB.tile([128, 8 * NJ], F32)
            gaz = cpool.tile([128, 8 * NJ], F32)
            for q in range(BC // 512):
                s = slice(512 * q, 512 * (q + 1))
                pX = psA.tile([HID, 512], F32, tag="pX")
                nc.tensor.matmul(pX[:], f1wT, stateT[:, s])
                nc.scalar.activation(xT[:, s], pX[:], AF.Tanh, bias=f1b)
            for q in range(BC // 512):
                s = slice(512 * q, 512 * (q + 1))
                pA = psA.tile([2 * N, 512], F32, tag="pA")
                nc.tensor.matmul(pA[:], wA, xT[:, s])
                nc.scalar.activation(wTall[:, s], pA[:], AF.Identity, bias=wAb)
            for j in range(NJ):
                nc.tensor.matmul(
                    pB[:, 8 * j : 8 * j + 8],
                    xT[:, 128 * j : 128 * (j + 1)],
                    oB,
                )
                if j % 4 == 3:
                    q = j // 4
                    nc.scalar.activation(
                        gaz[:, 32 * q : 32 * (q + 1)], pB[:, 32 * q : 32 * (q + 1)],
                        AF.Copy,
                    )
            gz = gaz[:].rearrange("p (j i) -> p i j", i=8)  # [128, 8, NJ]

            # ---------- phase 5: per-channel scan constants ----------
            az_t = cpool.tile([128, NJ], F32)
            lam_t = cpool.tile([128, NJ], F32)
            cq_t = cpool.tile([128, NJ], F32)
            s_t = cpool.tile([128, NJ], F32)
            tmp_t = cpool.tile([128, NJ], F32)
            cgy_t = cpool.tile([128, 2 * NJ], F32)
            e0b_t = cpool.tile([128, 2 * NJ], F32)

            # az = clip(az_raw + b_az, 0.5, 30)
            nc.vector.tensor_scalar(az_t[:], gz[:, 2, :], misc[:, 2:3], 0.5, AL.add, AL.max)
            nc.vector.tensor_scalar_min(az_t[:], az_t[:], 30.0)
            # lam = 1 - DT/2 * az
            nc.vector.tensor_scalar(lam_t[:], az_t[:], -DT / 2.0, 1.0, AL.mult, AL.add)
            # cq = az^2 / 4    (= abz)
            nc.vector.tensor_tensor(cq_t[:], az_t[:], az_t[:], AL.mult)
            nc.vector.tensor_scalar_mul(cq_t[:], cq_t[:], 0.25)
            # s = (lam - 1) * zs0 = -DT*zs0/2 * az
            nc.vector.tensor_scalar_mul(s_t[:], az_t[:], -DT * ZS0 / 2.0)
            for d in range(2):
                s = slice(NJ * d, NJ * (d + 1))
                # cgy = DT^2 * (g_raw + b_g - y0)
                nc.vector.tensor_tensor(cgy_t[:, s], gz[:, d, :], y0_sb[:, s], AL.subtract)
                nc.vector.tensor_scalar(
                    cgy_t[:, s], cgy_t[:, s], misc[:, d : d + 1], DT * DT, AL.add, AL.mult
                )
                # e0b = (lam-1)*zs0 + abz*cgy
                nc.vector.tensor_tensor(tmp_t[:], cq_t[:], cgy_t[:, s], AL.mult)
                nc.vector.tensor_tensor(e0b_t[:, s], tmp_t[:], s_t[:], AL.add)

            # ---------- phase 6: 32 chunks of [128 channels x 300 steps] ----------
            for d in range(2):
                for jg in range(NJ // NG):
                    Zb = zpool.tile([128, NG * T], F32, tag="Zb")
                    for js in range(NG):
                        j = jg * NG + js
                        k = NJ * d + j
                        zcol = js * T
                        pF = psF.tile([128, T], F32)
                        nc.tensor.matmul(
                            pF[:],
                            wTall[N * d : N * (d + 1), 128 * j : 128 * (j + 1)],
                            phiX_sb[N * d : N * (d + 1), :],
                        )
                        # G = cgy * [F_0 | dF_0..dF_297]; col0 also gets +e0b -> e_0
                        Gb = wpool.tile([128, T - 1], F32, tag="G")
                        nc.scalar.activation(
                            Gb[:, 0:1], pF[:, 0:1], AF.Identity,
                            bias=e0b_t[:, k : k + 1], scale=cgy_t[:, k : k + 1],
                        )
                        nc.scalar.activation(
                            Gb[:, 1 : T - 1], pF[:, 1 : T - 1], AF.Copy,
                            bias=0.0, scale=cgy_t[:, k : k + 1],
                        )
                        # e-scan: e_i = lam*e_{i-1} + G_{i-1}, e_0 from col0
                        E = wpool.tile([128, T - 1], F32, tag="E")
                        nc.vector.tensor_tensor_scan(
                            E[:],
                            lam_t[:, j : j + 1].broadcast_to([128, T - 1]),
                            Gb[:],
                            0.0,
                            AL.mult,
                            AL.add,
                        )
                        # zs-scan: zs_t = lam*zs_{t-1} + e_{t-1}, zs_0 = ZS0
                        nc.gpsimd.memset(Zb[:, zcol : zcol + 1], ZS0)
                        nc.vector.tensor_tensor_scan(
                            Zb[:, zcol + 1 : zcol + T],
                            lam_t[:, j : j + 1].broadcast_to([128, T - 1]),
                            E[:],
                            ZS0,
                            AL.mult,
                            AL.add,
                        )
                    nc.sync.dma_start(d_z.ap()[d, jg], Zb[:])

            # ---------- sigma / value outputs (off the critical path) ----------
            sig_s = wpool.tile([128, 2 * NJ], F32, tag="sv")
            val_s = wpool.tile([128, 2 * NJ], F32, tag="sv2")
            for d in range(2):
                s = slice(NJ * d, NJ * (d + 1))
                nc.scalar.activation(
                    sig_s[:, s], gz[:, 3 + d, :], AF.Sigmoid,
                    bias=misc[:, 3 + d : 4 + d],
                )
                nc.scalar.activation(
                    val_s[:, s], gz[:, 5 + d, :], AF.Identity,
                    bias=misc[:, 5 + d : 6 + d],
                )
            sig_o = wpool.tile([128, 2 * NJ], F32, tag="sv3")
            nc.vector.tensor_scalar_add(sig_o[:], sig_s[:], 0.001)
            nc.sync.dma_start(d_sig.ap(), sig_o[:])
            nc.sync.dma_start(d_val.ap(), val_s[:])

    nc.compile()
    return nc


def _phi_matrix(dmp_c, dmp_sigma2):
    """Host-precompute [N, T] matrix: col0 = Phi(0), col j = Phi(j)-Phi(j-1)
    for j=1..T-2, col T-1 = 0 (pad for even matmul moving dim).

    Phi(t)[n] = psi_t[n] * x_{t+1} / sum_n psi_t[n], x evolved in fp32 exactly
    as the reference does.
    """
    c = dmp_c.astype(np.float64)
    s2 = dmp_sigma2.astype(np.float64)
    xs = np.empty(T, np.float64)
    x = np.float32(1.0)
    dt32 = np.float32(DT)
    for t in range(T):
        x = np.float32(x + np.float32(np.float32(-x) * dt32))
        xs[t] = np.float64(x)
    psi = np.exp(-0.5 * (xs[:, None] - c[None, :]) ** 2 / s2[None, :])  # [T, N]
    phi = psi * xs[:, None] / psi.sum(axis=1, keepdims=True)            # [T, N]
    out = np.zeros((N, T), np.float64)
    out[:, 0] = phi[0]
    out[:, 1 : T - 1] = (phi[1 : T - 1] - phi[0 : T - 2]).T
    return out.astype(np.float32)


def kernel(state, fc1_w, fc1_b, fc2m_w, fc2m_b, sig_w, sig_b, val_w, val_b,
           dmp_c, dmp_sigma2):
    state = np.asarray(state, np.float32)
    fc1_w = np.asarray(fc1_w, np.float32)
    fc1_b = np.asarray(fc1_b, np.float32)
    fc2m_w = np.asarray(fc2m_w, np.float32)
    fc2m_b = np.asarray(fc2m_b, np.float32)
    sig_w = np.asarray(sig_w, np.float32)
    sig_b = np.asarray(sig_b, np.float32)
    val_w = np.asarray(val_w, np.float32)
    val_b = np.asarray(val_b, np.float32)
    dmp_c = np.asarray(dmp_c, np.float32)
    dmp_sigma2 = np.asarray(dmp_sigma2, np.float32)

    # ---- replicated host-prepped canvases ----
    canvas_r = np.zeros((64, CR_W), np.float32)
    canvas_r[:, CR_WA : CR_WA + 2 * N] = (0.1 * fc2m_w[2 : 2 + 2 * N]).T
    canvas_r[:, CR_OB : CR_OB + 7] = np.stack(
        [
            0.1 * fc2m_w[0], 0.1 * fc2m_w[1], 0.1 * fc2m_w[2 * N + 1],
            0.1 * sig_w[0], 0.1 * sig_w[1],
            0.1 * val_w[0], 0.1 * val_w[1],
        ],
        axis=1,
    )

    phiX = np.ascontiguousarray(np.tile(_phi_matrix(dmp_c, dmp_sigma2), (2, 1)))

    canvas_r[0:3, CR_F1W : CR_F1W + HID] = fc1_w.T

    canvas_f = np.zeros((128, CF_W), np.float32)
    canvas_f[0:HID, CF_F1B] = fc1_b
    canvas_f[0 : 2 * N, CF_WAB] = fc2m_b[2 : 2 + 2 * N]
    canvas_f[:, CF_MISC : CF_MISC + 8] = np.array(
        [fc2m_b[0], fc2m_b[1], fc2m_b[2 * N + 1],
         sig_b[0], sig_b[1], val_b[0], val_b[1], 0.0],
        np.float32,
    )

    in_maps = []
    for c in range(NCORES):
        sl = slice(c * BC, (c + 1) * BC)
        sc = state[sl]
        in_maps.append(
            {
                "stateT": np.ascontiguousarray(sc.T),
                "y0cols": np.ascontiguousarray(
                    sc[:, :2].reshape(NJ, 128, 2).transpose(1, 2, 0).reshape(128, 2 * NJ)
                ),
                "canvas_r": canvas_r,
                "canvas_f": canvas_f,
                "phiX": phiX,
            }
        )

    if "nc" not in _CACHE:
        _CACHE["nc"] = _build()
    nc = _CACHE["nc"]

    trace = bool(int(os.environ.get("CLAUDE_KERNEL_TRACE", "0")))
    if trace:
        _install_ntff_hook()
    res = run_bass_kernel_spmd(
        nc, in_maps, core_ids=list(range(NCORES)), trace=trace
    )
    _CACHE["exec_time_ns"] = res.exec_time_ns
    _CACHE["res"] = res

    # ---- gather / unshard ----
    a = np.empty((B, 2, T), np.float32)
    sig = np.empty((B, 2, 1), np.float32)
    val = np.empty((B, 2), np.float32)
    for c in range(NCORES):
        r = res.results[c]
        sl = slice(c * BC, (c + 1) * BC)
        # z: [2, NJ//NG, 128, NG*T] -> (d, jg, p, js, t) -> (jg, js, p, d, t)
        z = r["z"].reshape(2, NJ // NG, 128, NG, T)
        a[sl] = z.transpose(1, 3, 2, 0, 4).reshape(BC, 2, T)
        sig[sl, :, 0] = (
            r["sig"].reshape(128, 2, NJ).transpose(2, 0, 1).reshape(BC, 2)
        )
        val[sl] = r["val"].reshape(128, 2, NJ).transpose(2, 0, 1).reshape(BC, 2)
    value = np.ascontiguousarray(np.broadcast_to(val[:, :, None], (B, 2, T)))
    return a, sig, value


# revision 17
# speedup vs baseline: 1.3233x; 1.0891x over previous
"""Trainium2 Bass kernel for nn_CNNndpPolicy (CNN-NDP policy head + DMP rollout).

Strategy
--------
Pure data parallel over batch: each of the 8 NeuronCores processes B/8 = 2048
batch rows (M/8 = 4096 DMP channels).

The expensive part of the reference is a 300-step Euler rollout of the DMP
transformation system. Key algebraic facts exploited here:

1. The phase x_t = (1-DT)^t is input-independent, so the RBF basis
   Phi_t = psi(x_t) * x_t / sum(psi(x_t)) is a [300, 32] constant computed on
   the host. The forcing term for all steps is ONE matmul F = w @ Phi^T.
2. The output `a` is a first difference of Y, and y_{t+1} - y_t = DT*z_t,
   so a[:, t] = zs_t with zs = DT*z. Y itself is never materialized.
3. Eliminating y yields a second-order linear recurrence in zs:
       zs_{t+2} = 2*lam*zs_{t+1} - lam^2*zs_t + (U_{t+1} - U_t)
   with lam = 1 - DT*a_z/2 (critically damped: b_z = a_z/4 makes the
   characteristic discriminant exactly zero). This factors into TWO
   first-order scans (e-scan, zs-scan), each a single native
   `tensor_tensor_scan` DVE instruction per [128, 300] tile.
4. G_t = U_{t+1} - U_t = DT^2*(goal-y0)*(F_{t+1}-F_t), so the matmul is done
   directly against the differenced basis dPhi (host-precomputed), and the
   a_z*goal term cancels.

Engine budget per core: the two scans per 128-channel chunk are the hard
floor (~43us on the vector engine, ~2.2 cycles/element feedback rate);
everything else (fp32r matmuls on PE, PSUM->SBUF scaled copies on the scalar
engine, batched DMA) is arranged to hide underneath it.
"""

import os
import numpy as np

import concourse.bacc as bacc
import concourse.mybir as mybir
import concourse.tile as tile
from concourse.bass_utils import run_bass_kernel_spmd

# ---- problem constants (hardcoded per contract) ----
B, N, T, DIM, HID = 16384, 32, 300, 2, 64
NCORES = 8
BC = B // NCORES            # 2048 batch rows per core
NJ = BC // 128              # 16 partition blocks per core
NG = 4                      # chunks per output DMA group
DT = float(np.float32(1.0) / np.float32(300.0))
ZS0 = float(np.float32(0.01) * np.float32(DT))   # zs_0 = DT * z0
F32 = mybir.dt.float32
F32R = mybir.dt.float32r
AF = mybir.ActivationFunctionType
AL = mybir.AluOpType

# canvas_r (fp32r, [64, 136]): wA cols 0:64, oB cols 64:72 (7 used + pad),
# f1wT cols 72:136 (rows 0:3)
CR_WA, CR_OB, CR_F1W, CR_W = 0, 64, 72, 136
# canvas_f (fp32, [128, 11]): f1b col 0, wAb col 1, misc cols 3:11
CF_F1B, CF_WAB, CF_MISC, CF_W = 0, 1, 3, 11

_CACHE = {}


def _install_ntff_hook():
    """antenv.axon_hooks is missing in this image; shim it so trace=True works."""
    import sys
    import types

    try:
        from antenv.axon_hooks import get_axon_ntff_profile_hook  # noqa: F401
        return
    except ImportError:
        pass
    try:
        import antenv
        from trn_agent_boot.trn_boot import _ntff_profile_via_ctypes
    except ImportError:
        return
    mod = types.ModuleType("antenv.axon_hooks")
    _h = _ntff_profile_via_ctypes("/opt/axon/libaxon_pjrt.so")
    mod.get_axon_ntff_profile_hook = lambda: _h
    mod.set_axon_ntff_profile_hook = lambda h: None
    sys.modules["antenv.axon_hooks"] = mod
    antenv.axon_hooks = mod


def _build():
    nc = bacc.Bacc("TRN2", target_bir_lowering=False, debug=False)

    d_stateT = nc.dram_tensor("stateT", [3, BC], F32R, kind="ExternalInput")
    d_y0 = nc.dram_tensor("y0cols", [128, 2 * NJ], F32, kind="ExternalInput")
    d_cr = nc.dram_tensor("canvas_r", [64, CR_W], F32R, kind="ExternalInput")
    d_phiX = nc.dram_tensor("phiX", [2 * N, T], F32R, kind="ExternalInput")
    d_cf = nc.dram_tensor("canvas_f", [128, CF_W], F32, kind="ExternalInput")

    d_z = nc.dram_tensor("z", [2, NJ // NG, 128, NG * T], F32, kind="ExternalOutput")
    d_sig = nc.dram_tensor("sig", [128, 2 * NJ], F32, kind="ExternalOutput")
    d_val = nc.dram_tensor("val", [128, 2 * NJ], F32, kind="ExternalOutput")

    with tile.TileContext(nc) as tc:
        with (
            tc.tile_pool(name="const", bufs=1) as cpool,
            tc.tile_pool(name="psA", bufs=2, space="PSUM") as psA,
            tc.tile_pool(name="psB", bufs=2, space="PSUM") as psB,
            tc.tile_pool(name="psF", bufs=4, space="PSUM") as psF,
            tc.tile_pool(name="work", bufs=3) as wpool,
            tc.tile_pool(name="zout", bufs=3) as zpool,
        ):
            # ---------- input loads (stateT first: phase 1 needs it) ----------
            stateT = cpool.tile([3, BC], F32R)
            cf = cpool.tile([128, CF_W], F32)
            cr = cpool.tile([64, CR_W], F32R)
            y0_sb = cpool.tile([128, 2 * NJ], F32)
            phiX_sb = cpool.tile([2 * N, T], F32R)
            nc.sync.dma_start(stateT[:], d_stateT.ap())
            nc.sync.dma_start(cr[:], d_cr.ap())
            nc.gpsimd.dma_start(cf[:], d_cf.ap())
            nc.gpsimd.dma_start(y0_sb[:], d_y0.ap())
            nc.gpsimd.dma_start(phiX_sb[:], d_phiX.ap())

            f1b = cf[0:HID, CF_F1B : CF_F1B + 1]
            wAb = cf[0:2 * N, CF_WAB : CF_WAB + 1]
            misc = cf[:, CF_MISC : CF_MISC + 8]
            f1wT = cr[0:3, CR_F1W : CR_F1W + HID]
            wA = cr[:, CR_WA : CR_WA + 2 * N]
            oB = cr[:, CR_OB : CR_OB + 8]

            # persistent tiles of the head phases
            xT = cpool.tile([HID, BC], F32R)
            wTall = cpool.tile([2 * N, BC], F32R)
            gaz = cpool.tile([128, 8 * NJ], F32)
            gz = gaz[:].rearrange("p (j i) -> p i j", i=8)  # [128, 8, NJ]
            az_t = cpool.tile([128, NJ], F32)
            lam_t = cpool.tile([128, NJ], F32)
            cq_t = cpool.tile([128, NJ], F32)
            s_t = cpool.tile([128, NJ], F32)
            tmp_t = cpool.tile([128, NJ], F32)
            cgy_t = cpool.tile([128, 2 * NJ], F32)
            e0b_t = cpool.tile([128, 2 * NJ], F32)

            def phases_q(q):
                """Emit phase-1/2/3 + const-prep work for 512-batch block q.

                phase 1: xT[:, q] = tanh(fc1_w @ state^T + b)    [64, 512]
                phase 2: wTall[:, q] = (0.1*fc2m_w[2:66]) @ x^T + b  (rows 0:32
                         = d0 weights^T, 32:64 = d1; the chunk matmul pairs
                         wTall[32d:...] with row-duplicated phiX at the same
                         base partition)
                phase 3: orientation B heads, batch on partitions; cols per
                         block j: [g0, g1, az_raw, sig0, sig1, val0, val1, pad]
                const-prep: per-channel scan constants for j-block group q
                """
                s = slice(512 * q, 512 * (q + 1))
                pX = psA.tile([HID, 512], F32, tag="ph12", name=f"pX{q}")
                nc.tensor.matmul(pX[:], f1wT, stateT[:, s])
                nc.scalar.activation(xT[:, s], pX[:], AF.Tanh, bias=f1b)
                pA = psA.tile([2 * N, 512], F32, tag="ph12", name=f"pA{q}")
                nc.tensor.matmul(pA[:], wA, xT[:, s])
                nc.scalar.activation(wTall[:, s], pA[:], AF.Identity, bias=wAb)
                pBq = psB.tile([128, 32], F32, tag="pB", name=f"pB{q}")
                for jj in range(4):
                    nc.tensor.matmul(
                        pBq[:, 8 * jj : 8 * jj + 8],
                        xT[:, 512 * q + 128 * jj : 512 * q + 128 * (jj + 1)],
                        oB,
                    )
                nc.scalar.activation(
                    gaz[:, 32 * q : 32 * (q + 1)], pBq[:], AF.Copy
                )
                # const-prep for j in [4q, 4q+4): az/lam/cq/s + per-d cgy/e0b
                js = slice(4 * q, 4 * (q + 1))
                nc.vector.tensor_scalar(
                    az_t[:, js], gz[:, 2, js], misc[:, 2:3], 0.5, AL.add, AL.max
                )
                nc.vector.tensor_scalar_min(az_t[:, js], az_t[:, js], 30.0)
                nc.vector.tensor_scalar(
                    lam_t[:, js], az_t[:, js], -DT / 2.0, 1.0, AL.mult, AL.add
                )
                nc.vector.tensor_tensor(cq_t[:, js], az_t[:, js], az_t[:, js], AL.mult)
                nc.vector.tensor_scalar_mul(cq_t[:, js], cq_t[:, js], 0.25)
                nc.vector.tensor_scalar_mul(s_t[:, js], az_t[:, js], -DT * ZS0 / 2.0)
                for d in range(2):
                    ds = slice(NJ * d + 4 * q, NJ * d + 4 * (q + 1))
                    nc.vector.tensor_tensor(
                        cgy_t[:, ds], gz[:, d, js], y0_sb[:, ds], AL.subtract
                    )
                    nc.vector.tensor_scalar(
                        cgy_t[:, ds], cgy_t[:, ds], misc[:, d : d + 1], DT * DT,
                        AL.add, AL.mult,
                    )
                    nc.vector.tensor_tensor(tmp_t[:, js], cq_t[:, js], cgy_t[:, ds], AL.mult)
                    nc.vector.tensor_tensor(e0b_t[:, ds], tmp_t[:, js], s_t[:, js], AL.add)

            def chunk_group(d, jg):
                """4 chunks of [128 channels x 300 steps] + one batched DMA out."""
                Zb = zpool.tile([128, NG * T], F32, tag="Zb", name=f"Zb{d}_{jg}")
                for js_i in range(NG):
                    j = jg * NG + js_i
                    k = NJ * d + j
                    zcol = js_i * T
                    pF = psF.tile([128, T], F32, tag="pF", name=f"pF{d}_{j}")
                    nc.tensor.matmul(
                        pF[:],
                        wTall[N * d : N * (d + 1), 128 * j : 128 * (j + 1)],
                        phiX_sb[N * d : N * (d + 1), :],
                    )
                    # G = cgy * [F_0 | dF_0..dF_297]; col0 also gets +e0b -> e_0
                    Gb = wpool.tile([128, T - 1], F32, tag="G", name=f"G{d}_{j}")
                    nc.scalar.activation(
                        Gb[:, 0:1], pF[:, 0:1], AF.Identity,
                        bias=e0b_t[:, k : k + 1], scale=cgy_t[:, k : k + 1],
                    )
                    nc.scalar.activation(
                        Gb[:, 1 : T - 1], pF[:, 1 : T - 1], AF.Copy,
                        bias=0.0, scale=cgy_t[:, k : k + 1],
                    )
                    # e-scan: e_i = lam*e_{i-1} + G_{i-1}, e_0 from col0
                    E = wpool.tile([128, T - 1], F32, tag="E", name=f"E{d}_{j}")
                    nc.vector.tensor_tensor_scan(
                        E[:],
                        lam_t[:, j : j + 1].broadcast_to([128, T - 1]),
                        Gb[:],
                        0.0,
                        AL.mult,
                        AL.add,
                    )
                    # zs-scan: zs_t = lam*zs_{t-1} + e_{t-1}, zs_0 = ZS0
                    nc.gpsimd.memset(Zb[:, zcol : zcol + 1], ZS0)
                    nc.vector.tensor_tensor_scan(
                        Zb[:, zcol + 1 : zcol + T],
                        lam_t[:, j : j + 1].broadcast_to([128, T - 1]),
                        E[:],
                        ZS0,
                        AL.mult,
                        AL.add,
                    )
                nc.sync.dma_start(d_z.ap()[d, jg], Zb[:])

            def sig_val():
                """sigma / value outputs (hidden in the scan shadow)."""
                sig_s = wpool.tile([128, 2 * NJ], F32, tag="sv")
                val_s = wpool.tile([128, 2 * NJ], F32, tag="sv2")
                for d in range(2):
                    s = slice(NJ * d, NJ * (d + 1))
                    nc.scalar.activation(
                        sig_s[:, s], gz[:, 3 + d, :], AF.Sigmoid,
                        bias=misc[:, 3 + d : 4 + d],
                    )
                    nc.scalar.activation(
                        val_s[:, s], gz[:, 5 + d, :], AF.Identity,
                        bias=misc[:, 5 + d : 6 + d],
                    )
                sig_o = wpool.tile([128, 2 * NJ], F32, tag="sv3")
                nc.vector.tensor_scalar_add(sig_o[:], sig_s[:], 0.001)
                nc.sync.dma_start(d_sig.ap(), sig_o[:])
                nc.sync.dma_start(d_val.ap(), val_s[:])

            # ---------- software-pipelined emission ----------
            # chunks for j-group q-1 fill PE/ACT while block q's phases wait
            # on the previous engine in the chain
            phases_q(0)
            for q in range(1, BC // 512):
                phases_q(q)
                chunk_group(0, q - 1)
            chunk_group(0, NJ // NG - 1)
            chunk_group(1, 0)
            sig_val()
            for jg in range(1, NJ // NG):
                chunk_group(1, jg)

    nc.compile()
    return nc


def _phi_matrix(dmp_c, dmp_sigma2):
    """Host-precompute [N, T] matrix: col0 = Phi(0), col j = Phi(j)-Phi(j-1)
    for j=1..T-2, col T-1 = 0 (pad for even matmul moving dim).

    Phi(t)[n] = psi_t[n] * x_{t+1} / sum_n psi_t[n], x evolved in fp32 exactly
    as the reference does.
    """
    c = dmp_c.astype(np.float64)
    s2 = dmp_sigma2.astype(np.float64)
    xs = np.empty(T, np.float64)
    x = np.float32(1.0)
    dt32 = np.float32(DT)
    for t in range(T):
        x = np.float32(x + np.float32(np.float32(-x) * dt32))
        xs[t] = np.float64(x)
    psi = np.exp(-0.5 * (xs[:, None] - c[None, :]) ** 2 / s2[None, :])  # [T, N]
    phi = psi * xs[:, None] / psi.sum(axis=1, keepdims=True)            # [T, N]
    out = np.zeros((N, T), np.float64)
    out[:, 0] = phi[0]
    out[:, 1 : T - 1] = (phi[1 : T - 1] - phi[0 : T - 2]).T
    return out.astype(np.float32)


def kernel(state, fc1_w, fc1_b, fc2m_w, fc2m_b, sig_w, sig_b, val_w, val_b,
           dmp_c, dmp_sigma2):
    state = np.asarray(state, np.float32)
    fc1_w = np.asarray(fc1_w, np.float32)
    fc1_b = np.asarray(fc1_b, np.float32)
    fc2m_w = np.asarray(fc2m_w, np.float32)
    fc2m_b = np.asarray(fc2m_b, np.float32)
    sig_w = np.asarray(sig_w, np.float32)
    sig_b = np.asarray(sig_b, np.float32)
    val_w = np.asarray(val_w, np.float32)
    val_b = np.asarray(val_b, np.float32)
    dmp_c = np.asarray(dmp_c, np.float32)
    dmp_sigma2 = np.asarray(dmp_sigma2, np.float32)

    # ---- replicated host-prepped canvases ----
    canvas_r = np.zeros((64, CR_W), np.float32)
    canvas_r[:, CR_WA : CR_WA + 2 * N] = (0.1 * fc2m_w[2 : 2 + 2 * N]).T
    canvas_r[:, CR_OB : CR_OB + 7] = np.stack(
        [
            0.1 * fc2m_w[0], 0.1 * fc2m_w[1], 0.1 * fc2m_w[2 * N + 1],
            0.1 * sig_w[0], 0.1 * sig_w[1],
            0.1 * val_w[0], 0.1 * val_w[1],
        ],
        axis=1,
    )

    phiX = np.ascontiguousarray(np.tile(_phi_matrix(dmp_c, dmp_sigma2), (2, 1)))

    canvas_r[0:3, CR_F1W : CR_F1W + HID] = fc1_w.T

    canvas_f = np.zeros((128, CF_W), np.float32)
    canvas_f[0:HID, CF_F1B] = fc1_b
    canvas_f[0 : 2 * N, CF_WAB] = fc2m_b[2 : 2 + 2 * N]
    canvas_f[:, CF_MISC : CF_MISC + 8] = np.array(
        [fc2m_b[0], fc2m_b[1], fc2m_b[2 * N + 1],
         sig_b[0], sig_b[1], val_b[0], val_b[1], 0.0],
        np.float32,
    )

    in_maps = []
    for c in range(NCORES):
        sl = slice(c * BC, (c + 1) * BC)
        sc = state[sl]
        in_maps.append(
            {
                "stateT": np.ascontiguousarray(sc.T),
                "y0cols": np.ascontiguousarray(
                    sc[:, :2].reshape(NJ, 128, 2).transpose(1, 2, 0).reshape(128, 2 * NJ)
                ),
                "canvas_r": canvas_r,
                "canvas_f": canvas_f,
                "phiX": phiX,
            }
        )

    if "nc" not in _CACHE:
        _CACHE["nc"] = _build()
    nc = _CACHE["nc"]

    trace = bool(int(os.environ.get("CLAUDE_KERNEL_TRACE", "0")))
    if trace:
        _install_ntff_hook()
    res = run_bass_kernel_spmd(
        nc, in_maps, core_ids=list(range(NCORES)), trace=trace
    )
    _CACHE["exec_time_ns"] = res.exec_time_ns
    _CACHE["res"] = res

    # ---- gather / unshard ----
    a = np.empty((B, 2, T), np.float32)
    sig = np.empty((B, 2, 1), np.float32)
    val = np.empty((B, 2), np.float32)
    for c in range(NCORES):
        r = res.results[c]
        sl = slice(c * BC, (c + 1) * BC)
        # z: [2, NJ//NG, 128, NG*T] -> (d, jg, p, js, t) -> (jg, js, p, d, t)
        z = r["z"].reshape(2, NJ // NG, 128, NG, T)
        a[sl] = z.transpose(1, 3, 2, 0, 4).reshape(BC, 2, T)
        sig[sl, :, 0] = (
            r["sig"].reshape(128, 2, NJ).transpose(2, 0, 1).reshape(BC, 2)
        )
        val[sl] = r["val"].reshape(128, 2, NJ).transpose(2, 0, 1).reshape(BC, 2)
    value = np.ascontiguousarray(np.broadcast_to(val[:, :, None], (B, 2, T)))
    return a, sig, value
